# revision 17
# baseline (speedup 1.0000x reference)
"""Trainium2 Bass kernel for nn_NNModel2 (2x NNConv GNN + pooled MLP readout).

Self-contained: accepts FULL inputs, returns the FULL [256, 1] output.

Sharding: one balanced node-ownership map (512 nodes/core, chosen so that both
per-core edge counts are ~1024). conv1 uses dst-sharded edges with host-staged
x[src] tables; conv2 uses src-sharded edges so h1[src] is a local device
gather. conv2's output feeds only the linear graph mean-pool, so its messages
are scattered directly into transposed per-graph partial sums; lin1 is applied
to the (rec-scaled) partials before the collective, so a single bf16 AllReduce
of [128, 256] combines cores. No AllGather anywhere.

Math (per NNConv layer, aggr='add'):
    w_e  = (edge_attr @ nn_w + nn_b).reshape(E, I, O)
    msg  = einsum('ei,eio->eo', x[src], w_e)
restructured as a dense matmul over z[e,(k,i)] = attr[e,k] * x[src[e],i]:
    msg = z @ W' + x[src] @ B'
Scatter-add (nodes for conv1, graphs for conv2) via one-hot matmuls.
"""

import sys

sys.path.insert(0, "/opt/trn_rl_repo")

import numpy as np

from concourse import bacc, bass, mybir
import concourse.tile as tile
from concourse import bass_utils

P = 128
NCORES = 8
N_NODES = 4096
N_EDGES = 8192
N_GRAPHS = 256
DN = 64
DE = 32
H = 256
NSH = N_NODES // NCORES  # 512
NT = NSH // P  # 4
GT = N_GRAPHS // P  # 2

F32 = mybir.dt.float32
BF16 = mybir.dt.bfloat16
F16 = mybir.dt.float16
I16 = mybir.dt.int16
AF = mybir.ActivationFunctionType
ALU = mybir.AluOpType

_cache = {}


def _wrap_idx(idx, n):
    idx = np.asarray(idx, dtype=np.int16)
    assert idx.shape == (n,) and n % 16 == 0
    return np.tile(idx.reshape(n // 16, 16).T, (8, 1)).copy()


def _build(e_padA, e_padB, upto="full"):
    ETA = e_padA // P
    ETB = e_padB // P
    nc = bacc.Bacc(num_devices=NCORES)

    # ---- external inputs (host-prepped layouts, bf16 where possible)
    xsrcT_in = nc.dram_tensor("xsrcT_in", [P, e_padA], BF16, kind="ExternalInput")
    xshT_in = nc.dram_tensor("xshT_in", [DN, NSH], BF16, kind="ExternalInput")
    attrAe_in = nc.dram_tensor("attrAe_in", [16, e_padA], BF16, kind="ExternalInput")
    attrAo_in = nc.dram_tensor("attrAo_in", [16, e_padA], BF16, kind="ExternalInput")
    attrB_in = nc.dram_tensor("attrB_in", [DE, e_padB], BF16, kind="ExternalInput")
    w1p = nc.dram_tensor("w1p", [P, 16, H], BF16, kind="ExternalInput")
    b1p_in = nc.dram_tensor("b1p_in", [DN, H], BF16, kind="ExternalInput")
    r1w_in = nc.dram_tensor("r1w_in", [DN, H], BF16, kind="ExternalInput")
    b1row_in = nc.dram_tensor("b1row_in", [1, H], BF16, kind="ExternalInput")
    w2p = nc.dram_tensor("w2p", [P, 64, H], BF16, kind="ExternalInput")
    b2p_in = nc.dram_tensor("b2p_in", [P, 2, H], BF16, kind="ExternalInput")
    r2w_in = nc.dram_tensor("r2w_in", [P, 2, H], BF16, kind="ExternalInput")
    b2colT_in = nc.dram_tensor("b2colT_in", [P, 2, 1], BF16, kind="ExternalInput")
    l1w_in = nc.dram_tensor("l1w_in", [P, 2, H // 2], BF16, kind="ExternalInput")
    l1b_in = nc.dram_tensor("l1b_in", [H // 2, 1], F32, kind="ExternalInput")
    l2w_in = nc.dram_tensor("l2w_in", [H // 2, 1], BF16, kind="ExternalInput")
    l2b_in = nc.dram_tensor("l2b_in", [1, 1], F32, kind="ExternalInput")
    io512_in = nc.dram_tensor("io512_in", [P, NSH], F16, kind="ExternalInput")
    iotag_in = nc.dram_tensor("iotag_in", [P, N_GRAPHS], F16, kind="ExternalInput")
    recrow_in = nc.dram_tensor("recrow_in", [1, N_GRAPHS], BF16, kind="ExternalInput")
    mask_in = nc.dram_tensor("mask_in", [1, N_GRAPHS], BF16, kind="ExternalInput")
    # per-core index tables
    srcB_w = nc.dram_tensor("srcB_w", [P, e_padB // 16], I16, kind="ExternalInput")
    id512_w = nc.dram_tensor("id512_w", [P, NSH // 16], I16, kind="ExternalInput")
    dstlA_in = nc.dram_tensor("dstlA_in", [e_padA, 1], F32, kind="ExternalInput")
    gdstB_in = nc.dram_tensor("gdstB_in", [e_padB, 1], F32, kind="ExternalInput")
    batchl_in = nc.dram_tensor("batchl_in", [NSH, 1], F32, kind="ExternalInput")
    out = nc.dram_tensor("out", [N_GRAPHS, 1], F32, kind="ExternalOutput")

    def dbg_out(name, shape):
        return nc.dram_tensor(name, shape, F32, kind="ExternalOutput")

    rg = [list(range(NCORES))]
    ST = {"gather": 1, "msg1": 2, "h1": 3, "msg2": 4, "pool": 5, "full": 99}[upto]

    with tile.TileContext(nc, num_cores=NCORES) as tc:
        with (
            tc.tile_pool(name="const", bufs=1) as cp,
            tc.tile_pool(name="work", bufs=3) as wp,
            tc.tile_pool(name="dram", bufs=1, space="DRAM") as dr,
        ):
            # ======== stage 0: conv1-critical loads first
            xsrcT = cp.tile([P, e_padA], BF16)
            bcA = cp.tile([P, 16, e_padA], BF16)
            w1sb = cp.tile([P, 16, H], BF16)

            def bca_chunk(c):
                sl = slice(4 * c, 4 * (c + 1))
                nc.sync.dma_start(
                    out=bcA[0:DN, sl, :],
                    in_=attrAe_in[sl, :].partition_broadcast(DN),
                )
                nc.sync.dma_start(
                    out=bcA[DN:P, sl, :],
                    in_=attrAo_in[sl, :].partition_broadcast(DN),
                )

            with tc.high_priority():
                nc.sync.dma_start(out=xsrcT[:], in_=xsrcT_in[:])
                bca_chunk(0)
                nc.sync.dma_start(out=w1sb[:], in_=w1p[:])
                for c in range(1, 4):
                    bca_chunk(c)
            b1p = cp.tile([DN, H], BF16)
            nc.sync.dma_start(out=b1p[:], in_=b1p_in[:])
            r1wb = cp.tile([DN, H], BF16)
            nc.sync.dma_start(out=r1wb[:], in_=r1w_in[:])
            b1row = cp.tile([1, H], BF16)
            nc.sync.dma_start(out=b1row[:], in_=b1row_in[:])
            io512 = cp.tile([P, NSH], F16)
            nc.sync.dma_start(out=io512[:], in_=io512_in[:])
            dstlA_sb = cp.tile([P, ETA, 1], F32)
            nc.sync.dma_start(
                out=dstlA_sb[:], in_=dstlA_in.rearrange("(e p) one -> p e one", p=P)
            )
            xshT = cp.tile([DN, NSH], BF16)
            nc.sync.dma_start(out=xshT[:], in_=xshT_in[:])
            ones = cp.tile([1, P], BF16)
            nc.vector.memset(ones[:], 1.0)

            # ---- conv2 weights + broadcast: stream during conv1
            srcB_sb = cp.tile([P, e_padB // 16], I16)
            nc.sync.dma_start(out=srcB_sb[:], in_=srcB_w[:])
            id512_sb = cp.tile([P, NSH // 16], I16)
            nc.sync.dma_start(out=id512_sb[:], in_=id512_w[:])
            w2sb = cp.tile([P, 64, H], BF16)
            bcB = cp.tile([P, DE, e_padB], BF16)
            # small consts
            iotag = cp.tile([P, N_GRAPHS], F16)
            nc.sync.dma_start(out=iotag[:], in_=iotag_in[:])
            gdstl_sb = cp.tile([P, ETB, 1], F32)
            nc.sync.dma_start(
                out=gdstl_sb[:], in_=gdstB_in.rearrange("(e p) one -> p e one", p=P)
            )
            batchl_sb = cp.tile([P, NT, 1], F32)
            nc.sync.dma_start(
                out=batchl_sb[:], in_=batchl_in.rearrange("(t p) one -> p t one", p=P)
            )
            r2wb = cp.tile([P, 2, H], BF16)
            nc.sync.dma_start(out=r2wb[:], in_=r2w_in[:])
            b2p = cp.tile([P, 2, H], BF16)
            nc.sync.dma_start(out=b2p[:], in_=b2p_in[:])
            b2colT = cp.tile([P, 2, 1], BF16)
            nc.sync.dma_start(out=b2colT[:], in_=b2colT_in[:])
            l1wb = cp.tile([P, 2, H // 2], BF16)
            nc.sync.dma_start(out=l1wb[:], in_=l1w_in[:])
            l1bsb = cp.tile([H // 2, 1], F32)
            nc.sync.dma_start(out=l1bsb[:], in_=l1b_in[:])
            l2wb = cp.tile([H // 2, 1], BF16)
            nc.sync.dma_start(out=l2wb[:], in_=l2w_in[:])
            l2bsb = cp.tile([1, 1], F32)
            nc.sync.dma_start(out=l2bsb[:], in_=l2b_in[:])
            recbc = cp.tile([P, N_GRAPHS], BF16)
            nc.sync.dma_start(
                out=recbc[:], in_=recrow_in[0:1, :].partition_broadcast(P)
            )
            masksb = cp.tile([1, N_GRAPHS], BF16)
            nc.sync.dma_start(out=masksb[:], in_=mask_in[:])

            def bcb_dma(c, eng):
                eng.dma_start(
                    out=bcB[:, 8 * c : 8 * (c + 1), :],
                    in_=attrB_in[8 * c : 8 * (c + 1), :].partition_broadcast(P),
                )

            def w2_dma(c, eng):
                eng.dma_start(
                    out=w2sb[:, 16 * c : 16 * (c + 1), :],
                    in_=w2p[:, 16 * c : 16 * (c + 1), :],
                )

            with tc.tile_wait_until(0.012):
                bcb_dma(0, nc.gpsimd)
                w2_dma(0, nc.gpsimd)
            with tc.tile_wait_until(0.021):
                bcb_dma(1, nc.gpsimd)
                w2_dma(1, nc.gpsimd)

            if upto == "gather":
                d = dbg_out("d_xsrcT", [P, e_padA])
                tmp = wp.tile([P, e_padA], F32, tag="dbgf")
                nc.vector.tensor_copy(out=tmp[:], in_=xsrcT[:])
                nc.sync.dma_start(out=d[:], in_=tmp[:])
                d2 = dbg_out("d_bcB5", [P, e_padB])
                tmp2 = wp.tile([P, e_padB], F32, tag="dbgf")
                nc.vector.tensor_copy(out=tmp2[:], in_=bcB[:, 5, :])
                nc.sync.dma_start(out=d2[:], in_=tmp2[:])

            with tc.tile_pool(name="psA", bufs=1, space="PSUM") as psA:
                # ======== conv1: msg1 = z1 @ W1' + x_src @ B1'
                msg_ps = [
                    psA.tile([P, 2 * H], F32, space="PSUM",
                             tag=f"msg{j}", name=f"msg1_{j}")
                    for j in range((ETA + 1) // 2)
                ]

                def m1(e):
                    return msg_ps[e // 2][:, (e % 2) * H : (e % 2) * H + H]

                if ST >= 2:
                    for t in range(16):
                        zt = wp.tile([P, e_padA], BF16, tag="zt", bufs=4)
                        nc.vector.tensor_tensor(
                            out=zt[:], in0=xsrcT[:], in1=bcA[:, t, :], op=ALU.mult
                        )
                        for e in range(ETA):
                            nc.tensor.matmul(
                                m1(e), lhsT=zt[:, P * e : P * (e + 1)],
                                rhs=w1sb[:, t, :],
                                start=(t == 0 and e % 2 == 0), stop=False,
                                skip_group_check=True,
                            )
                    for e in range(ETA):
                        nc.tensor.matmul(
                            m1(e), lhsT=xsrcT[0:DN, P * e : P * (e + 1)],
                            rhs=b1p[:], start=False, stop=True,
                            skip_group_check=True,
                        )

                    msbs = []
                    for j in range((ETA + 1) // 2):
                        w = min(2 * H, (ETA - 2 * j) * H)
                        msb = wp.tile([P, 2 * H], BF16, tag="msb")
                        nc.scalar.activation(
                            out=msb[:, 0:w], in_=msg_ps[j][:, 0:w], func=AF.Copy
                        )
                        msbs.append(msb)

                if upto == "msg1":
                    dm = dbg_out("d_msg1", [P, ETA * H])
                    for j in range((ETA + 1) // 2):
                        w = min(2 * H, (ETA - 2 * j) * H)
                        tmpm = wp.tile([P, 2 * H], F32, tag="dbgm")
                        nc.vector.tensor_copy(out=tmpm[:, 0:w], in_=msbs[j][:, 0:w])
                        nc.sync.dma_start(
                            out=dm[:, 2 * H * j : 2 * H * j + w], in_=tmpm[:, 0:w])

                # ---- conv1 scatter to own nodes + root + bias, relu
                agg_ps = [
                    psA.tile([P, 2 * H], F32, space="PSUM",
                             tag=f"agg{j}", name=f"agg1_{j}")
                    for j in range(NT // 2)
                ]

                def a1(n):
                    return agg_ps[n // 2][:, (n % 2) * H : (n % 2) * H + H]

                if ST >= 3:
                    for e in range(ETA):
                        for n in range(NT):
                            oh = wp.tile([P, P], BF16, tag="oh", bufs=6)
                            nc.vector.tensor_scalar(
                                out=oh[:], in0=io512[:, P * n : P * (n + 1)],
                                scalar1=dstlA_sb[:, e, :1], scalar2=None,
                                op0=ALU.is_equal,
                            )
                            nc.tensor.matmul(
                                a1(n), lhsT=oh[:],
                                rhs=msbs[e // 2][:, (e % 2) * H : (e % 2) * H + H],
                                start=(e == 0 and n % 2 == 0), stop=False,
                                skip_group_check=True,
                            )
                    for n in range(NT):
                        nc.tensor.matmul(
                            a1(n), lhsT=xshT[:, P * n : P * (n + 1)],
                            rhs=r1wb[:], start=False, stop=False,
                            skip_group_check=True,
                        )
                        nc.tensor.matmul(
                            a1(n), lhsT=ones[:], rhs=b1row[:],
                            start=False, stop=True, skip_group_check=True,
                        )
                    h1sb = cp.tile([P, NT, H], BF16)
                    for j in range(NT // 2):
                        nc.scalar.activation(
                            out=h1sb[:, 2 * j : 2 * j + 2, :],
                            in_=agg_ps[j][:, 0 : 2 * H], func=AF.Relu,
                        )
                    h1loc = dr.tile([NSH, H], BF16)
                    nc.sync.dma_start(
                        out=h1loc[:].rearrange("(t p) o -> p t o", p=P), in_=h1sb[:]
                    )

                    # local gathers for conv2
                    h1srcT = cp.tile([P, 2, e_padB], BF16)
                    nc.gpsimd.dma_gather(
                        out_ap=h1srcT[:], in_ap=h1loc[:], idxs_ap=srcB_sb[:],
                        num_idxs=e_padB, num_idxs_reg=e_padB, elem_size=H,
                        transpose=True, single_packet=False,
                    )
                    h1ownT = cp.tile([P, 2, NSH], BF16)
                    nc.gpsimd.dma_gather(
                        out_ap=h1ownT[:], in_ap=h1loc[:], idxs_ap=id512_sb[:],
                        num_idxs=NSH, num_idxs_reg=NSH, elem_size=H,
                        transpose=True, single_packet=False,
                    )
                    with tc.tile_wait_until(0.036):
                        for c in range(2, 4):
                            bcb_dma(c, nc.gpsimd)
                            w2_dma(c, nc.gpsimd)


                if upto == "h1":
                    dh = dbg_out("d_h1", [P, NT * H])
                    tmp = wp.tile([P, NT, H], F32, tag="dbgf")
                    nc.vector.tensor_copy(out=tmp[:], in_=h1sb[:])
                    nc.sync.dma_start(
                        out=dh[:].rearrange("p (t o) -> p t o", o=H), in_=tmp[:]
                    )

                # ---- early small matmul: b2l1 = (b2/8) @ l1w (for bias2 fold)
                with tc.tile_pool(name="psS", bufs=1, space="PSUM") as psS:
                    b2l1_ps = psS.tile([1, H // 2], F32, space="PSUM", tag="b2l1")
                    for kh in range(2):
                        nc.tensor.matmul(
                            b2l1_ps[:], lhsT=b2colT[:, kh, :1], rhs=l1wb[:, kh, :],
                            start=(kh == 0), stop=(kh == 1),
                        )
                    b2l1sb = cp.tile([1, H // 2], BF16)
                    nc.scalar.activation(out=b2l1sb[:], in_=b2l1_ps[:], func=AF.Copy)

                # ======== conv2: msg2 = z2 @ W2' + h1_src @ B2'
                if ST >= 4:
                    msg2_ps = [
                        psA.tile([P, 2 * H], F32, space="PSUM",
                                 tag=f"msg{j}", name=f"msg2_{j}")
                        for j in range((ETB + 1) // 2)
                    ]

                    def m2(e):
                        return msg2_ps[e // 2][:, (e % 2) * H : (e % 2) * H + H]

                    for t in range(64):
                        k, ih = t // 2, t % 2
                        zt = wp.tile([P, e_padB], BF16, tag="zt", bufs=4)
                        nc.vector.tensor_tensor(
                            out=zt[:], in0=h1srcT[:, ih, :], in1=bcB[:, k, :],
                            op=ALU.mult,
                        )
                        for e in range(ETB):
                            nc.tensor.matmul(
                                m2(e), lhsT=zt[:, P * e : P * (e + 1)],
                                rhs=w2sb[:, t, :],
                                start=(t == 0 and e % 2 == 0), stop=False,
                                skip_group_check=True,
                            )
                    for e in range(ETB):
                        for ih in range(2):
                            nc.tensor.matmul(
                                m2(e), lhsT=h1srcT[:, ih, P * e : P * (e + 1)],
                                rhs=b2p[:, ih, :], start=False,
                                stop=(ih == 1), skip_group_check=True,
                            )

                    msbs2 = []
                    for j in range((ETB + 1) // 2):
                        w = min(2 * H, (ETB - 2 * j) * H)
                        msb = wp.tile([P, 2 * H], BF16, tag="msb")
                        nc.scalar.activation(
                            out=msb[:, 0:w], in_=msg2_ps[j][:, 0:w], func=AF.Copy
                        )
                        msbs2.append(msb)

                if upto == "msg2":
                    dm = dbg_out("d_msg2", [P, ETB * H])
                    for j in range((ETB + 1) // 2):
                        w = min(2 * H, (ETB - 2 * j) * H)
                        tmpm = wp.tile([P, 2 * H], F32, tag="dbgm")
                        nc.vector.tensor_copy(out=tmpm[:, 0:w], in_=msbs2[j][:, 0:w])
                        nc.sync.dma_start(
                            out=dm[:, 2 * H * j : 2 * H * j + w], in_=tmpm[:, 0:w])

                # ---- conv2 root transform r2 = h1_own @ root2_w
                if ST >= 4:
                    r2_ps = [
                        psA.tile([P, 2 * H], F32, space="PSUM",
                                 tag=f"agg{j}", name=f"r2_{j}")
                        for j in range(NT // 2)
                    ]
                    for n in range(NT):
                        for kh in range(2):
                            nc.tensor.matmul(
                                r2_ps[n // 2][:, (n % 2) * H : (n % 2) * H + H],
                                lhsT=h1ownT[:, kh, P * n : P * (n + 1)],
                                rhs=r2wb[:, kh, :],
                                start=(kh == 0), stop=(kh == 1),
                                skip_group_check=True,
                            )
                    r2sb = cp.tile([P, NT, H], BF16)
                    for j in range(NT // 2):
                        nc.scalar.activation(
                            out=r2sb[:, 2 * j : 2 * j + 2, :],
                            in_=r2_ps[j][:, 0 : 2 * H], func=AF.Copy,
                        )

                    # ---- transposed scatter into poolT[o, g]:
                    #   poolT[o, g] += sum_e msg2[e, o]*[gdst(e)=g]
                    #                + sum_n r2[n, o]*[batch(n)=g]
                    pool_t = psA.tile([P, 2 * H], F32, space="PSUM",
                                      tag="pool", name="poolT")
                    for e in range(ETB):
                        ohg = wp.tile([P, N_GRAPHS], BF16, tag="ohg", bufs=4)
                        nc.vector.tensor_scalar(
                            out=ohg[:], in0=iotag[:],
                            scalar1=gdstl_sb[:, e, :1], scalar2=None,
                            op0=ALU.is_equal,
                        )
                        for hh in range(2):
                            nc.tensor.matmul(
                                pool_t[:, hh * H : hh * H + H],
                                lhsT=msbs2[e // 2][
                                    :, (e % 2) * H + hh * P : (e % 2) * H + hh * P + P
                                ],
                                rhs=ohg[:],
                                start=(e == 0 and hh == 0), stop=False,
                                skip_group_check=True,
                            )
                    for n in range(NT):
                        ohb = wp.tile([P, N_GRAPHS], BF16, tag="ohg", bufs=4)
                        nc.vector.tensor_scalar(
                            out=ohb[:], in0=iotag[:],
                            scalar1=batchl_sb[:, n, :1], scalar2=None,
                            op0=ALU.is_equal,
                        )
                        for hh in range(2):
                            nc.tensor.matmul(
                                pool_t[:, hh * H : hh * H + H],
                                lhsT=r2sb[:, n, hh * P : hh * P + P],
                                rhs=ohb[:],
                                start=False, stop=(n == NT - 1 and hh == 1),
                                skip_group_check=True,
                            )
                    # poolT psum -> bf16, scale by 1/cnt (per graph, free axis)
                    plsb = cp.tile([P, 2, N_GRAPHS], BF16)
                    nc.scalar.activation(
                        out=plsb[:], in_=pool_t[:, 0 : 2 * H], func=AF.Copy
                    )
                    scl = cp.tile([P, 2, N_GRAPHS], BF16)
                    for hh in range(2):
                        nc.vector.tensor_tensor(
                            out=scl[:, hh, :], in0=plsb[:, hh, :], in1=recbc[:],
                            op=ALU.mult,
                        )
                    # z1 partial = scl^T @ l1w + (b2/8 @ l1w) x mask
                    z1p_t = psA.tile([P, 2 * H], F32, space="PSUM",
                                     tag="pool", name="z1p")
                    z1p = z1p_t[:, 0:N_GRAPHS]
                    for hh in range(2):
                        nc.tensor.matmul(
                            z1p, lhsT=l1wb[:, hh, :], rhs=scl[:, hh, :],
                            start=(hh == 0), stop=False, skip_group_check=True,
                        )
                    nc.tensor.matmul(
                        z1p, lhsT=b2l1sb[:], rhs=masksb[:],
                        start=False, stop=True, skip_group_check=True,
                    )
                    z1psb = cp.tile([H // 2, N_GRAPHS], BF16)
                    nc.scalar.activation(out=z1psb[:], in_=z1p, func=AF.Copy)
                    pcc_in = dr.tile([H // 2, N_GRAPHS], BF16)
                    nc.sync.dma_start(out=pcc_in[:], in_=z1psb[:])
                    pcc_out = dr.tile([H // 2, N_GRAPHS], BF16, addr_space="Shared")
                    nc.gpsimd.collective_compute(
                        "AllReduce", ALU.add, replica_groups=rg,
                        ins=[pcc_in[:].opt()], outs=[pcc_out[:].opt()],
                    )

                if upto == "pool":
                    dr2 = dbg_out("d_r2", [P, NT * H])
                    tmp2 = wp.tile([P, NT, H], F32, tag="dbgr")
                    nc.vector.tensor_copy(out=tmp2[:], in_=r2sb[:])
                    nc.sync.dma_start(
                        out=dr2[:].rearrange("p (t o) -> p t o", o=H), in_=tmp2[:]
                    )
                    dp = dbg_out("d_poolT", [P, 2 * N_GRAPHS])
                    tmp = wp.tile([P, 2, N_GRAPHS], F32, tag="dbgf")
                    nc.vector.tensor_copy(out=tmp[:], in_=plsb[:])
                    nc.sync.dma_start(
                        out=dp[:].rearrange("p (h g) -> p h g", g=N_GRAPHS), in_=tmp[:]
                    )
                    dz = dbg_out("d_z1p", [H // 2, N_GRAPHS])
                    tmpz = wp.tile([H // 2, N_GRAPHS], F32, tag="dbgz")
                    nc.vector.tensor_copy(out=tmpz[:], in_=z1psb[:])
                    nc.sync.dma_start(out=dz[:], in_=tmpz[:])

            # ======== tail: readout MLP (redundant on every core)
            if ST >= 5 and upto == "full":
                with tc.tile_pool(name="psB", bufs=1, space="PSUM") as psB:
                    pl = cp.tile([H // 2, N_GRAPHS], BF16)
                    nc.sync.dma_start(out=pl[:], in_=pcc_out[:])
                    z1sb = cp.tile([H // 2, N_GRAPHS], BF16)
                    nc.scalar.activation(
                        out=z1sb[:], in_=pl[:], func=AF.Relu, bias=l1bsb[:, :1]
                    )
                    o_ps = psB.tile([1, N_GRAPHS], F32, space="PSUM", tag="op")
                    nc.tensor.matmul(
                        o_ps[:], lhsT=l2wb[:], rhs=z1sb[:], start=True, stop=True
                    )
                    osb = cp.tile([1, N_GRAPHS], F32)
                    nc.scalar.activation(
                        out=osb[:], in_=o_ps[:], func=AF.Sigmoid, bias=l2bsb[:, :1]
                    )
                    nc.sync.dma_start(
                        out=out[:].rearrange("g one -> one g"), in_=osb[:]
                    )

    nc.compile()
    return nc


def _balance_owner(src, dst):
    """Assign 512 nodes/core s.t. per-core indeg and outdeg sums are balanced."""
    indeg = np.bincount(dst, minlength=N_NODES)
    outdeg = np.bincount(src, minlength=N_NODES)
    order = np.argsort(-(indeg + outdeg), kind="stable")
    owner = np.full(N_NODES, -1, dtype=np.int64)
    in_load = np.zeros(NCORES, dtype=np.int64)
    out_load = np.zeros(NCORES, dtype=np.int64)
    slots = np.full(NCORES, NSH, dtype=np.int64)
    for n in order:
        best, bkey = -1, None
        for c in range(NCORES):
            if slots[c] == 0:
                continue
            key = (
                max(in_load[c] + indeg[n], out_load[c] + outdeg[n]),
                in_load[c] + out_load[c],
            )
            if bkey is None or key < bkey:
                best, bkey = c, key
        owner[n] = best
        in_load[best] += indeg[n]
        out_load[best] += outdeg[n]
        slots[best] -= 1
    return owner, int(in_load.max()), int(out_load.max())


def _bf16(a):
    import ml_dtypes

    return np.asarray(np.asarray(a, np.float32), dtype=ml_dtypes.bfloat16)


def _prep_inputs(inputs):
    x = np.asarray(inputs["x"], dtype=np.float32)
    ei = np.asarray(inputs["edge_index"])
    attr = np.asarray(inputs["edge_attr"], dtype=np.float32)
    batch = np.asarray(inputs["batch"]).astype(np.int64)
    src, dst = ei[0].astype(np.int64), ei[1].astype(np.int64)

    owner, max_in, max_out = _balance_owner(src, dst)
    e_padA = max(((max_in + P - 1) // P) * P, P)
    e_padB = max(((max_out + P - 1) // P) * P, P)

    own = [np.nonzero(owner == c)[0] for c in range(NCORES)]
    local_id = np.zeros(N_NODES, dtype=np.int64)
    for c in range(NCORES):
        local_id[own[c]] = np.arange(NSH)

    nn1_w = np.asarray(inputs["nn1_w"], dtype=np.float32)  # [32, 64*256]
    nn2_w = np.asarray(inputs["nn2_w"], dtype=np.float32)  # [32, 256*256]
    nn1_b = np.asarray(inputs["nn1_b"], dtype=np.float32)
    nn2_b = np.asarray(inputs["nn2_b"], dtype=np.float32)

    # w1p[p, t, o] = nn1_w[2t + p//64, (p%64)*256 + o]
    w1r = nn1_w.reshape(16, 2, DN, H)  # [t, k2, i, o]
    w1p = np.ascontiguousarray(w1r.transpose(1, 2, 0, 3).reshape(P, 16, H))
    # w2p[p, t, o] = nn2_w[t//2, ((t%2)*128 + p)*256 + o]
    w2r = nn2_w.reshape(DE, 2, P, H)  # [k, ih, p, o]
    w2p = np.ascontiguousarray(w2r.transpose(2, 0, 1, 3).reshape(P, 64, H))
    b2pr = nn2_b.reshape(2, P, H).transpose(1, 0, 2)  # [p, ih, o]

    cnt = np.bincount(batch, minlength=N_GRAPHS).astype(np.float32)
    recrow = (1.0 / np.maximum(cnt, 1.0)).reshape(1, N_GRAPHS)
    maskrow = (cnt > 0).astype(np.float32).reshape(1, N_GRAPHS)

    r2w = np.asarray(inputs["root2_w"], dtype=np.float32)  # [256, 256]
    b2 = np.asarray(inputs["bias2"], dtype=np.float32)  # [256]
    l1w = np.asarray(inputs["lin1_w"], dtype=np.float32)  # [256, 128]

    x_bf = _bf16(x)
    attr_bf = _bf16(attr)

    common = {
        "w1p": _bf16(w1p),
        "b1p_in": _bf16(nn1_b.reshape(DN, H)),
        "r1w_in": _bf16(np.asarray(inputs["root1_w"], np.float32)),
        "b1row_in": _bf16(np.asarray(inputs["bias1"], np.float32).reshape(1, H)),
        "w2p": _bf16(w2p),
        "b2p_in": _bf16(b2pr),
        "r2w_in": _bf16(r2w.reshape(2, P, H).transpose(1, 0, 2)),
        "b2colT_in": _bf16((b2 / NCORES).reshape(2, P, 1).transpose(1, 0, 2)),
        "l1w_in": _bf16(l1w.reshape(2, P, H // 2).transpose(1, 0, 2)),
        "l1b_in": np.asarray(inputs["lin1_b"], np.float32).reshape(-1, 1),
        "l2w_in": _bf16(np.asarray(inputs["lin2_w"], np.float32)),
        "l2b_in": np.asarray(inputs["lin2_b"], np.float32).reshape(1, 1),
        "io512_in": np.tile(np.arange(NSH, dtype=np.float16), (P, 1)),
        "iotag_in": np.tile(np.arange(N_GRAPHS, dtype=np.float16), (P, 1)),
        "recrow_in": _bf16(recrow),
        "mask_in": _bf16(maskrow),
        "id512_w": _wrap_idx(np.arange(NSH, dtype=np.int16), NSH),
    }

    in_maps = []
    for c in range(NCORES):
        eA = np.nonzero(owner[dst] == c)[0]
        eB = np.nonzero(owner[src] == c)[0]
        nA, nB = len(eA), len(eB)
        assert nA <= e_padA and nB <= e_padB

        # host-staged gather tables for conv1 (bf16)
        xsrcA = np.zeros((P, e_padA), dtype=x_bf.dtype)
        xsrcA[0:DN, :nA] = x_bf[src[eA]].T
        xsrcA[DN:P, :nA] = x_bf[src[eA]].T
        attrA = np.zeros((DE, e_padA), dtype=attr_bf.dtype)
        attrA[:, :nA] = attr_bf[eA].T
        attrB = np.zeros((DE, e_padB), dtype=attr_bf.dtype)
        attrB[:, :nB] = attr_bf[eB].T
        xsh = np.ascontiguousarray(x_bf[own[c]].T)  # [64, 512]

        dstlA = np.full(e_padA, -1.0, dtype=np.float32)
        dstlA[:nA] = local_id[dst[eA]].astype(np.float32)
        srcB = np.zeros(e_padB, dtype=np.int16)
        srcB[:nB] = local_id[src[eB]]
        gdstB = np.full(e_padB, -1.0, dtype=np.float32)
        gdstB[:nB] = batch[dst[eB]].astype(np.float32)

        m = dict(common)
        m["xsrcT_in"] = xsrcA
        m["xshT_in"] = xsh
        m["attrAe_in"] = np.ascontiguousarray(attrA[0::2, :])
        m["attrAo_in"] = np.ascontiguousarray(attrA[1::2, :])
        m["attrB_in"] = attrB
        m["srcB_w"] = _wrap_idx(srcB, e_padB)
        m["dstlA_in"] = dstlA.reshape(-1, 1)
        m["gdstB_in"] = gdstB.reshape(-1, 1)
        m["batchl_in"] = batch[own[c]].astype(np.float32).reshape(-1, 1)
        in_maps.append(m)
    return (e_padA, e_padB), in_maps


def kernel(**inputs) -> np.ndarray:
    key, in_maps = _prep_inputs(inputs)
    if key not in _cache:
        _cache[key] = _build(*key)
    nc = _cache[key]
    res = bass_utils.run_bass_kernel_spmd(nc, in_maps, core_ids=list(range(NCORES)))
    return np.asarray(res.results[0]["out"], dtype=np.float32)


def run_debug(upto, **inputs):
    key, in_maps = _prep_inputs(inputs)
    nc = _build(*key, upto=upto)
    res = bass_utils.run_bass_kernel_spmd(nc, in_maps, core_ids=list(range(NCORES)))
    return key, res


# revision 18
# speedup vs baseline: 1.2950x; 1.2950x over previous
"""Trainium2 Bass kernel for nn_NNModel2 (2x NNConv GNN + pooled MLP readout).

Self-contained: accepts FULL inputs, returns the FULL [256, 1] output.

Sharding: one balanced node-ownership map (512 nodes/core, chosen so that both
per-core edge counts are ~1024). conv1 uses dst-sharded edges with host-staged
x[src] tables; conv2 uses src-sharded edges so h1[src] is a local device
gather. conv2's output feeds only the linear graph mean-pool, so its messages
are scattered directly into transposed per-graph partial sums; lin1 is applied
to the (rec-scaled) partials before the collective, so a single bf16 AllReduce
of [128, 256] combines cores. No AllGather anywhere.

Math (per NNConv layer, aggr='add'):
    w_e  = (edge_attr @ nn_w + nn_b).reshape(E, I, O)
    msg  = einsum('ei,eio->eo', x[src], w_e)
restructured as a dense matmul over z[e,(k,i)] = attr[e,k] * x[src[e],i]:
    msg = z @ W' + x[src] @ B'
Scatter-add (nodes for conv1, graphs for conv2) via one-hot matmuls.
"""

import sys

sys.path.insert(0, "/opt/trn_rl_repo")

import numpy as np

from concourse import bacc, bass, mybir
import concourse.tile as tile
from concourse import bass_utils

P = 128
NCORES = 8
N_NODES = 4096
N_EDGES = 8192
N_GRAPHS = 256
DN = 64
DE = 32
H = 256
NSH = N_NODES // NCORES  # 512
NT = NSH // P  # 4
GT = N_GRAPHS // P  # 2

F32 = mybir.dt.float32
BF16 = mybir.dt.bfloat16
F16 = mybir.dt.float16
I16 = mybir.dt.int16
AF = mybir.ActivationFunctionType
ALU = mybir.AluOpType

_cache = {}


def _wrap_idx(idx, n):
    idx = np.asarray(idx, dtype=np.int16)
    assert idx.shape == (n,) and n % 16 == 0
    return np.tile(idx.reshape(n // 16, 16).T, (8, 1)).copy()


def _build(e_padA, e_padB, upto="full"):
    ETA = e_padA // P
    ETB = e_padB // P
    nc = bacc.Bacc(num_devices=NCORES)

    # ---- external inputs (host-prepped layouts, bf16 where possible)
    xsrcT_in = nc.dram_tensor("xsrcT_in", [P, e_padA], BF16, kind="ExternalInput")
    xshT_in = nc.dram_tensor("xshT_in", [DN, NSH], BF16, kind="ExternalInput")
    attrAe_in = nc.dram_tensor("attrAe_in", [16, e_padA], BF16, kind="ExternalInput")
    attrAo_in = nc.dram_tensor("attrAo_in", [16, e_padA], BF16, kind="ExternalInput")
    attrB_in = nc.dram_tensor("attrB_in", [DE, e_padB], BF16, kind="ExternalInput")
    w1p = nc.dram_tensor("w1p", [P, 16, H], BF16, kind="ExternalInput")
    b1p_in = nc.dram_tensor("b1p_in", [DN, H], BF16, kind="ExternalInput")
    r1w_in = nc.dram_tensor("r1w_in", [DN, H], BF16, kind="ExternalInput")
    b1row_in = nc.dram_tensor("b1row_in", [1, H], BF16, kind="ExternalInput")
    w2p = nc.dram_tensor("w2p", [P, 64, H], BF16, kind="ExternalInput")
    b2p_in = nc.dram_tensor("b2p_in", [P, 2, H], BF16, kind="ExternalInput")
    r2w_in = nc.dram_tensor("r2w_in", [P, 2, H], BF16, kind="ExternalInput")
    b2colT_in = nc.dram_tensor("b2colT_in", [P, 2, 1], BF16, kind="ExternalInput")
    l1w_in = nc.dram_tensor("l1w_in", [P, 2, H // 2], BF16, kind="ExternalInput")
    l1b_in = nc.dram_tensor("l1b_in", [H // 2, 1], F32, kind="ExternalInput")
    l2w_in = nc.dram_tensor("l2w_in", [H // 2, 1], BF16, kind="ExternalInput")
    l2b_in = nc.dram_tensor("l2b_in", [1, 1], F32, kind="ExternalInput")
    io512_in = nc.dram_tensor("io512_in", [P, NSH], F16, kind="ExternalInput")
    iotag_in = nc.dram_tensor("iotag_in", [P, N_GRAPHS], F16, kind="ExternalInput")
    recrow_in = nc.dram_tensor("recrow_in", [1, N_GRAPHS], BF16, kind="ExternalInput")
    mask_in = nc.dram_tensor("mask_in", [1, N_GRAPHS], BF16, kind="ExternalInput")
    # per-core index tables
    srcbc_in = nc.dram_tensor("srcbc_in", [1, e_padB], F16, kind="ExternalInput")
    nodeio_in = nc.dram_tensor("nodeio_in", [P, NT], F32, kind="ExternalInput")
    ohid_in = nc.dram_tensor("ohid_in", [P, NT, NSH], BF16, kind="ExternalInput")
    dstlA_in = nc.dram_tensor("dstlA_in", [e_padA, 1], F32, kind="ExternalInput")
    gdstB_in = nc.dram_tensor("gdstB_in", [e_padB, 1], F32, kind="ExternalInput")
    batchl_in = nc.dram_tensor("batchl_in", [NSH, 1], F32, kind="ExternalInput")
    out = nc.dram_tensor("out", [N_GRAPHS, 1], F32, kind="ExternalOutput")

    def dbg_out(name, shape):
        return nc.dram_tensor(name, shape, F32, kind="ExternalOutput")

    rg = [list(range(NCORES))]
    ST = {"gather": 1, "msg1": 2, "h1": 3, "msg2": 4, "pool": 5, "full": 99}[upto]

    with tile.TileContext(nc, num_cores=NCORES) as tc:
        with (
            tc.tile_pool(name="const", bufs=1) as cp,
            tc.tile_pool(name="work", bufs=3) as wp,
            tc.tile_pool(name="dram", bufs=1, space="DRAM") as dr,
        ):
            # ======== stage 0: conv1-critical loads first
            xsrcT = cp.tile([P, e_padA], BF16)
            bcA = cp.tile([P, 16, e_padA], BF16)
            w1sb = cp.tile([P, 16, H], BF16)

            def bca_chunk(c):
                sl = slice(4 * c, 4 * (c + 1))
                nc.sync.dma_start(
                    out=bcA[0:DN, sl, :],
                    in_=attrAe_in[sl, :].partition_broadcast(DN),
                )
                nc.sync.dma_start(
                    out=bcA[DN:P, sl, :],
                    in_=attrAo_in[sl, :].partition_broadcast(DN),
                )

            with tc.high_priority():
                nc.sync.dma_start(out=xsrcT[:], in_=xsrcT_in[:])
                bca_chunk(0)
                nc.sync.dma_start(out=w1sb[:], in_=w1p[:])
                for c in range(1, 4):
                    bca_chunk(c)
            b1p = cp.tile([DN, H], BF16)
            nc.sync.dma_start(out=b1p[:], in_=b1p_in[:])
            r1wb = cp.tile([DN, H], BF16)
            nc.sync.dma_start(out=r1wb[:], in_=r1w_in[:])
            b1row = cp.tile([1, H], BF16)
            nc.sync.dma_start(out=b1row[:], in_=b1row_in[:])
            io512 = cp.tile([P, NSH], F16)
            nc.sync.dma_start(out=io512[:], in_=io512_in[:])
            dstlA_sb = cp.tile([P, ETA, 1], F32)
            nc.sync.dma_start(
                out=dstlA_sb[:], in_=dstlA_in.rearrange("(e p) one -> p e one", p=P)
            )
            xshT = cp.tile([DN, NSH], BF16)
            nc.sync.dma_start(out=xshT[:], in_=xshT_in[:])
            ones = cp.tile([1, P], BF16)
            nc.vector.memset(ones[:], 1.0)

            # ---- conv2 weights + broadcast: stream during conv1
            srcbc = cp.tile([P, e_padB], F16)
            nc.sync.dma_start(
                out=srcbc[:], in_=srcbc_in[0:1, :].partition_broadcast(P)
            )
            nodeio = cp.tile([P, NT], F32)
            nc.sync.dma_start(out=nodeio[:], in_=nodeio_in[:])
            ohid = cp.tile([P, NT, NSH], BF16)
            nc.sync.dma_start(out=ohid[:], in_=ohid_in[:])
            w2sb = cp.tile([P, 64, H], BF16)
            bcB = cp.tile([P, DE, e_padB], BF16)
            # small consts
            iotag = cp.tile([P, N_GRAPHS], F16)
            nc.sync.dma_start(out=iotag[:], in_=iotag_in[:])
            gdstl_sb = cp.tile([P, ETB, 1], F32)
            nc.sync.dma_start(
                out=gdstl_sb[:], in_=gdstB_in.rearrange("(e p) one -> p e one", p=P)
            )
            batchl_sb = cp.tile([P, NT, 1], F32)
            nc.sync.dma_start(
                out=batchl_sb[:], in_=batchl_in.rearrange("(t p) one -> p t one", p=P)
            )
            r2wb = cp.tile([P, 2, H], BF16)
            nc.sync.dma_start(out=r2wb[:], in_=r2w_in[:])
            b2p = cp.tile([P, 2, H], BF16)
            nc.sync.dma_start(out=b2p[:], in_=b2p_in[:])
            b2colT = cp.tile([P, 2, 1], BF16)
            nc.sync.dma_start(out=b2colT[:], in_=b2colT_in[:])
            l1wb = cp.tile([P, 2, H // 2], BF16)
            nc.sync.dma_start(out=l1wb[:], in_=l1w_in[:])
            l1bsb = cp.tile([H // 2, 1], F32)
            nc.sync.dma_start(out=l1bsb[:], in_=l1b_in[:])
            l2wb = cp.tile([H // 2, 1], BF16)
            nc.sync.dma_start(out=l2wb[:], in_=l2w_in[:])
            l2bsb = cp.tile([1, 1], F32)
            nc.sync.dma_start(out=l2bsb[:], in_=l2b_in[:])
            recbc = cp.tile([P, N_GRAPHS], BF16)
            nc.sync.dma_start(
                out=recbc[:], in_=recrow_in[0:1, :].partition_broadcast(P)
            )
            masksb = cp.tile([1, N_GRAPHS], BF16)
            nc.sync.dma_start(out=masksb[:], in_=mask_in[:])

            def bcb_dma(c, eng):
                eng.dma_start(
                    out=bcB[:, 8 * c : 8 * (c + 1), :],
                    in_=attrB_in[8 * c : 8 * (c + 1), :].partition_broadcast(P),
                )

            def w2_dma(c, eng):
                eng.dma_start(
                    out=w2sb[:, 16 * c : 16 * (c + 1), :],
                    in_=w2p[:, 16 * c : 16 * (c + 1), :],
                )

            for c in range(4):
                bcb_dma(c, nc.sync)
                w2_dma(c, nc.sync)

            if upto == "gather":
                d = dbg_out("d_xsrcT", [P, e_padA])
                tmp = wp.tile([P, e_padA], F32, tag="dbgf")
                nc.vector.tensor_copy(out=tmp[:], in_=xsrcT[:])
                nc.sync.dma_start(out=d[:], in_=tmp[:])
                d2 = dbg_out("d_bcB5", [P, e_padB])
                tmp2 = wp.tile([P, e_padB], F32, tag="dbgf")
                nc.vector.tensor_copy(out=tmp2[:], in_=bcB[:, 5, :])
                nc.sync.dma_start(out=d2[:], in_=tmp2[:])

            with tc.tile_pool(name="psA", bufs=1, space="PSUM") as psA:
                # ======== conv1: msg1 = z1 @ W1' + x_src @ B1'
                msg_ps = [
                    psA.tile([P, 2 * H], F32, space="PSUM",
                             tag=f"msg{j}", name=f"msg1_{j}")
                    for j in range((ETA + 1) // 2)
                ]

                def m1(e):
                    return msg_ps[e // 2][:, (e % 2) * H : (e % 2) * H + H]

                if ST >= 2:
                    for t in range(16):
                        zt = wp.tile([P, e_padA], BF16, tag="zt", bufs=4)
                        nc.vector.tensor_tensor(
                            out=zt[:], in0=xsrcT[:], in1=bcA[:, t, :], op=ALU.mult
                        )
                        for e in range(ETA):
                            nc.tensor.matmul(
                                m1(e), lhsT=zt[:, P * e : P * (e + 1)],
                                rhs=w1sb[:, t, :],
                                start=(t == 0 and e % 2 == 0), stop=False,
                                skip_group_check=True,
                            )
                    for e in range(ETA):
                        nc.tensor.matmul(
                            m1(e), lhsT=xsrcT[0:DN, P * e : P * (e + 1)],
                            rhs=b1p[:], start=False, stop=True,
                            skip_group_check=True,
                        )

                    msbs = []
                    for j in range((ETA + 1) // 2):
                        w = min(2 * H, (ETA - 2 * j) * H)
                        msb = wp.tile([P, 2 * H], BF16, tag="msb")
                        nc.scalar.activation(
                            out=msb[:, 0:w], in_=msg_ps[j][:, 0:w], func=AF.Copy
                        )
                        msbs.append(msb)

                if upto == "msg1":
                    dm = dbg_out("d_msg1", [P, ETA * H])
                    for j in range((ETA + 1) // 2):
                        w = min(2 * H, (ETA - 2 * j) * H)
                        tmpm = wp.tile([P, 2 * H], F32, tag="dbgm")
                        nc.vector.tensor_copy(out=tmpm[:, 0:w], in_=msbs[j][:, 0:w])
                        nc.sync.dma_start(
                            out=dm[:, 2 * H * j : 2 * H * j + w], in_=tmpm[:, 0:w])

                # ---- conv1 scatter to own nodes + root + bias, relu
                agg_ps = [
                    psA.tile([P, 2 * H], F32, space="PSUM",
                             tag=f"agg{j}", name=f"agg1_{j}")
                    for j in range(NT // 2)
                ]

                def a1(n):
                    return agg_ps[n // 2][:, (n % 2) * H : (n % 2) * H + H]

                if ST >= 3:
                    for e in range(ETA):
                        for n in range(NT):
                            oh = wp.tile([P, P], BF16, tag="oh", bufs=6)
                            nc.vector.tensor_scalar(
                                out=oh[:], in0=io512[:, P * n : P * (n + 1)],
                                scalar1=dstlA_sb[:, e, :1], scalar2=None,
                                op0=ALU.is_equal,
                            )
                            nc.tensor.matmul(
                                a1(n), lhsT=oh[:],
                                rhs=msbs[e // 2][:, (e % 2) * H : (e % 2) * H + H],
                                start=(e == 0 and n % 2 == 0), stop=False,
                                skip_group_check=True,
                            )
                    for n in range(NT):
                        nc.tensor.matmul(
                            a1(n), lhsT=xshT[:, P * n : P * (n + 1)],
                            rhs=r1wb[:], start=False, stop=False,
                            skip_group_check=True,
                        )
                        nc.tensor.matmul(
                            a1(n), lhsT=ones[:], rhs=b1row[:],
                            start=False, stop=True, skip_group_check=True,
                        )
                    h1sb = cp.tile([P, NT, H], BF16)
                    for j in range(NT // 2):
                        nc.scalar.activation(
                            out=h1sb[:, 2 * j : 2 * j + 2, :],
                            in_=agg_ps[j][:, 0 : 2 * H], func=AF.Relu,
                        )
                    # PE one-hot "gathers": h1srcT[i, e] = h1[srcB[e], i],
                    # h1ownT[i, n'] = h1[n', i]
                    ohsrc = []
                    for nt in range(NT):
                        o = wp.tile([P, e_padB], BF16, tag="ohsrc", bufs=4)
                        nc.vector.tensor_scalar(
                            out=o[:], in0=srcbc[:],
                            scalar1=nodeio[:, nt : nt + 1], scalar2=None,
                            op0=ALU.is_equal,
                        )
                        ohsrc.append(o)
                    h1srcT = cp.tile([P, 2, e_padB], BF16)
                    EH = (e_padB + 511) // 512
                    for ih in range(2):
                        for eh in range(EH):
                            w = min(512, e_padB - 512 * eh)
                            gps = psA.tile([P, 2 * H], F32, space="PSUM",
                                           tag=f"msg{(2 * ih + eh) % 4}",
                                           name=f"g{ih}_{eh}")
                            for nt in range(NT):
                                nc.tensor.matmul(
                                    gps[:, 0:w],
                                    lhsT=h1sb[:, nt, P * ih : P * ih + P],
                                    rhs=ohsrc[nt][:, 512 * eh : 512 * eh + w],
                                    start=(nt == 0), stop=(nt == NT - 1),
                                    skip_group_check=True,
                                )
                            nc.scalar.activation(
                                out=h1srcT[:, ih, 512 * eh : 512 * eh + w],
                                in_=gps[:, 0:w], func=AF.Copy,
                            )
                    h1ownT = cp.tile([P, 2, NSH], BF16)
                    for ih in range(2):
                        gps = psA.tile([P, 2 * H], F32, space="PSUM",
                                       tag=f"agg{ih}", name=f"go{ih}")
                        for nt in range(NT):
                            nc.tensor.matmul(
                                gps[:, 0:NSH],
                                lhsT=h1sb[:, nt, P * ih : P * ih + P],
                                rhs=ohid[:, nt, :],
                                start=(nt == 0), stop=(nt == NT - 1),
                                skip_group_check=True,
                            )
                        nc.scalar.activation(
                            out=h1ownT[:, ih, :], in_=gps[:, 0:NSH], func=AF.Copy
                        )


                if upto == "h1":
                    dh = dbg_out("d_h1", [P, NT * H])
                    tmp = wp.tile([P, NT, H], F32, tag="dbgf")
                    nc.vector.tensor_copy(out=tmp[:], in_=h1sb[:])
                    nc.sync.dma_start(
                        out=dh[:].rearrange("p (t o) -> p t o", o=H), in_=tmp[:]
                    )

                # ---- early small matmul: b2l1 = (b2/8) @ l1w (for bias2 fold)
                with tc.tile_pool(name="psS", bufs=1, space="PSUM") as psS:
                    b2l1_ps = psS.tile([1, H // 2], F32, space="PSUM", tag="b2l1")
                    for kh in range(2):
                        nc.tensor.matmul(
                            b2l1_ps[:], lhsT=b2colT[:, kh, :1], rhs=l1wb[:, kh, :],
                            start=(kh == 0), stop=(kh == 1),
                        )
                    b2l1sb = cp.tile([1, H // 2], BF16)
                    nc.scalar.activation(out=b2l1sb[:], in_=b2l1_ps[:], func=AF.Copy)

                # ======== conv2: msg2 = z2 @ W2' + h1_src @ B2'
                if ST >= 4:
                    msg2_ps = [
                        psA.tile([P, 2 * H], F32, space="PSUM",
                                 tag=f"msg{j}", name=f"msg2_{j}")
                        for j in range((ETB + 1) // 2)
                    ]

                    def m2(e):
                        return msg2_ps[e // 2][:, (e % 2) * H : (e % 2) * H + H]

                    for t in range(64):
                        k, ih = t // 2, t % 2
                        zt = wp.tile([P, e_padB], BF16, tag="zt", bufs=4)
                        nc.vector.tensor_tensor(
                            out=zt[:], in0=h1srcT[:, ih, :], in1=bcB[:, k, :],
                            op=ALU.mult,
                        )
                        for e in range(ETB):
                            nc.tensor.matmul(
                                m2(e), lhsT=zt[:, P * e : P * (e + 1)],
                                rhs=w2sb[:, t, :],
                                start=(t == 0 and e % 2 == 0), stop=False,
                                skip_group_check=True,
                            )
                    for e in range(ETB):
                        for ih in range(2):
                            nc.tensor.matmul(
                                m2(e), lhsT=h1srcT[:, ih, P * e : P * (e + 1)],
                                rhs=b2p[:, ih, :], start=False,
                                stop=(ih == 1), skip_group_check=True,
                            )

                    msbs2 = []
                    for j in range((ETB + 1) // 2):
                        w = min(2 * H, (ETB - 2 * j) * H)
                        msb = wp.tile([P, 2 * H], BF16, tag="msb")
                        nc.scalar.activation(
                            out=msb[:, 0:w], in_=msg2_ps[j][:, 0:w], func=AF.Copy
                        )
                        msbs2.append(msb)

                if upto == "msg2":
                    dm = dbg_out("d_msg2", [P, ETB * H])
                    for j in range((ETB + 1) // 2):
                        w = min(2 * H, (ETB - 2 * j) * H)
                        tmpm = wp.tile([P, 2 * H], F32, tag="dbgm")
                        nc.vector.tensor_copy(out=tmpm[:, 0:w], in_=msbs2[j][:, 0:w])
                        nc.sync.dma_start(
                            out=dm[:, 2 * H * j : 2 * H * j + w], in_=tmpm[:, 0:w])

                # ---- conv2 root transform r2 = h1_own @ root2_w
                if ST >= 4:
                    r2_ps = [
                        psA.tile([P, 2 * H], F32, space="PSUM",
                                 tag=f"agg{j}", name=f"r2_{j}")
                        for j in range(NT // 2)
                    ]
                    for n in range(NT):
                        for kh in range(2):
                            nc.tensor.matmul(
                                r2_ps[n // 2][:, (n % 2) * H : (n % 2) * H + H],
                                lhsT=h1ownT[:, kh, P * n : P * (n + 1)],
                                rhs=r2wb[:, kh, :],
                                start=(kh == 0), stop=(kh == 1),
                                skip_group_check=True,
                            )
                    r2sb = cp.tile([P, NT, H], BF16)
                    for j in range(NT // 2):
                        nc.scalar.activation(
                            out=r2sb[:, 2 * j : 2 * j + 2, :],
                            in_=r2_ps[j][:, 0 : 2 * H], func=AF.Copy,
                        )

                    # ---- transposed scatter into poolT[o, g]:
                    #   poolT[o, g] += sum_e msg2[e, o]*[gdst(e)=g]
                    #                + sum_n r2[n, o]*[batch(n)=g]
                    pool_t = psA.tile([P, 2 * H], F32, space="PSUM",
                                      tag="pool", name="poolT")
                    for e in range(ETB):
                        ohg = wp.tile([P, N_GRAPHS], BF16, tag="ohg", bufs=4)
                        nc.vector.tensor_scalar(
                            out=ohg[:], in0=iotag[:],
                            scalar1=gdstl_sb[:, e, :1], scalar2=None,
                            op0=ALU.is_equal,
                        )
                        for hh in range(2):
                            nc.tensor.matmul(
                                pool_t[:, hh * H : hh * H + H],
                                lhsT=msbs2[e // 2][
                                    :, (e % 2) * H + hh * P : (e % 2) * H + hh * P + P
                                ],
                                rhs=ohg[:],
                                start=(e == 0 and hh == 0), stop=False,
                                skip_group_check=True,
                            )
                    for n in range(NT):
                        ohb = wp.tile([P, N_GRAPHS], BF16, tag="ohg", bufs=4)
                        nc.vector.tensor_scalar(
                            out=ohb[:], in0=iotag[:],
                            scalar1=batchl_sb[:, n, :1], scalar2=None,
                            op0=ALU.is_equal,
                        )
                        for hh in range(2):
                            nc.tensor.matmul(
                                pool_t[:, hh * H : hh * H + H],
                                lhsT=r2sb[:, n, hh * P : hh * P + P],
                                rhs=ohb[:],
                                start=False, stop=(n == NT - 1 and hh == 1),
                                skip_group_check=True,
                            )
                    # poolT psum -> bf16, scale by 1/cnt (per graph, free axis)
                    plsb = cp.tile([P, 2, N_GRAPHS], BF16)
                    nc.scalar.activation(
                        out=plsb[:], in_=pool_t[:, 0 : 2 * H], func=AF.Copy
                    )
                    scl = cp.tile([P, 2, N_GRAPHS], BF16)
                    for hh in range(2):
                        nc.vector.tensor_tensor(
                            out=scl[:, hh, :], in0=plsb[:, hh, :], in1=recbc[:],
                            op=ALU.mult,
                        )
                    # z1 partial = scl^T @ l1w + (b2/8 @ l1w) x mask
                    z1p_t = psA.tile([P, 2 * H], F32, space="PSUM",
                                     tag="pool", name="z1p")
                    z1p = z1p_t[:, 0:N_GRAPHS]
                    for hh in range(2):
                        nc.tensor.matmul(
                            z1p, lhsT=l1wb[:, hh, :], rhs=scl[:, hh, :],
                            start=(hh == 0), stop=False, skip_group_check=True,
                        )
                    nc.tensor.matmul(
                        z1p, lhsT=b2l1sb[:], rhs=masksb[:],
                        start=False, stop=True, skip_group_check=True,
                    )
                    z1psb = cp.tile([H // 2, N_GRAPHS], BF16)
                    nc.scalar.activation(out=z1psb[:], in_=z1p, func=AF.Copy)
                    pcc_in = dr.tile([H // 2, N_GRAPHS], BF16)
                    nc.sync.dma_start(out=pcc_in[:], in_=z1psb[:])
                    pcc_out = dr.tile([H // 2, N_GRAPHS], BF16, addr_space="Shared")
                    nc.gpsimd.collective_compute(
                        "AllReduce", ALU.add, replica_groups=rg,
                        ins=[pcc_in[:].opt()], outs=[pcc_out[:].opt()],
                    )

                if upto == "pool":
                    dr2 = dbg_out("d_r2", [P, NT * H])
                    tmp2 = wp.tile([P, NT, H], F32, tag="dbgr")
                    nc.vector.tensor_copy(out=tmp2[:], in_=r2sb[:])
                    nc.sync.dma_start(
                        out=dr2[:].rearrange("p (t o) -> p t o", o=H), in_=tmp2[:]
                    )
                    dp = dbg_out("d_poolT", [P, 2 * N_GRAPHS])
                    tmp = wp.tile([P, 2, N_GRAPHS], F32, tag="dbgf")
                    nc.vector.tensor_copy(out=tmp[:], in_=plsb[:])
                    nc.sync.dma_start(
                        out=dp[:].rearrange("p (h g) -> p h g", g=N_GRAPHS), in_=tmp[:]
                    )
                    dz = dbg_out("d_z1p", [H // 2, N_GRAPHS])
                    tmpz = wp.tile([H // 2, N_GRAPHS], F32, tag="dbgz")
                    nc.vector.tensor_copy(out=tmpz[:], in_=z1psb[:])
                    nc.sync.dma_start(out=dz[:], in_=tmpz[:])

            # ======== tail: readout MLP (redundant on every core)
            if ST >= 5 and upto == "full":
                with tc.tile_pool(name="psB", bufs=1, space="PSUM") as psB:
                    pl = cp.tile([H // 2, N_GRAPHS], BF16)
                    nc.sync.dma_start(out=pl[:], in_=pcc_out[:])
                    z1sb = cp.tile([H // 2, N_GRAPHS], BF16)
                    nc.scalar.activation(
                        out=z1sb[:], in_=pl[:], func=AF.Relu, bias=l1bsb[:, :1]
                    )
                    o_ps = psB.tile([1, N_GRAPHS], F32, space="PSUM", tag="op")
                    nc.tensor.matmul(
                        o_ps[:], lhsT=l2wb[:], rhs=z1sb[:], start=True, stop=True
                    )
                    osb = cp.tile([1, N_GRAPHS], F32)
                    nc.scalar.activation(
                        out=osb[:], in_=o_ps[:], func=AF.Sigmoid, bias=l2bsb[:, :1]
                    )
                    nc.sync.dma_start(
                        out=out[:].rearrange("g one -> one g"), in_=osb[:]
                    )

    nc.compile()
    return nc


def _balance_owner(src, dst):
    """Assign 512 nodes/core s.t. per-core indeg and outdeg sums are balanced."""
    indeg = np.bincount(dst, minlength=N_NODES)
    outdeg = np.bincount(src, minlength=N_NODES)
    order = np.argsort(-(indeg + outdeg), kind="stable")
    owner = np.full(N_NODES, -1, dtype=np.int64)
    in_load = np.zeros(NCORES, dtype=np.int64)
    out_load = np.zeros(NCORES, dtype=np.int64)
    slots = np.full(NCORES, NSH, dtype=np.int64)
    for n in order:
        best, bkey = -1, None
        for c in range(NCORES):
            if slots[c] == 0:
                continue
            key = (
                max(in_load[c] + indeg[n], out_load[c] + outdeg[n]),
                in_load[c] + out_load[c],
            )
            if bkey is None or key < bkey:
                best, bkey = c, key
        owner[n] = best
        in_load[best] += indeg[n]
        out_load[best] += outdeg[n]
        slots[best] -= 1
    return owner, int(in_load.max()), int(out_load.max())


def _bf16(a):
    import ml_dtypes

    return np.asarray(np.asarray(a, np.float32), dtype=ml_dtypes.bfloat16)


def _prep_inputs(inputs):
    x = np.asarray(inputs["x"], dtype=np.float32)
    ei = np.asarray(inputs["edge_index"])
    attr = np.asarray(inputs["edge_attr"], dtype=np.float32)
    batch = np.asarray(inputs["batch"]).astype(np.int64)
    src, dst = ei[0].astype(np.int64), ei[1].astype(np.int64)

    owner, max_in, max_out = _balance_owner(src, dst)
    e_padA = max(((max_in + P - 1) // P) * P, P)
    e_padB = max(((max_out + P - 1) // P) * P, P)

    own = [np.nonzero(owner == c)[0] for c in range(NCORES)]
    local_id = np.zeros(N_NODES, dtype=np.int64)
    for c in range(NCORES):
        local_id[own[c]] = np.arange(NSH)

    nn1_w = np.asarray(inputs["nn1_w"], dtype=np.float32)  # [32, 64*256]
    nn2_w = np.asarray(inputs["nn2_w"], dtype=np.float32)  # [32, 256*256]
    nn1_b = np.asarray(inputs["nn1_b"], dtype=np.float32)
    nn2_b = np.asarray(inputs["nn2_b"], dtype=np.float32)

    # w1p[p, t, o] = nn1_w[2t + p//64, (p%64)*256 + o]
    w1r = nn1_w.reshape(16, 2, DN, H)  # [t, k2, i, o]
    w1p = np.ascontiguousarray(w1r.transpose(1, 2, 0, 3).reshape(P, 16, H))
    # w2p[p, t, o] = nn2_w[t//2, ((t%2)*128 + p)*256 + o]
    w2r = nn2_w.reshape(DE, 2, P, H)  # [k, ih, p, o]
    w2p = np.ascontiguousarray(w2r.transpose(2, 0, 1, 3).reshape(P, 64, H))
    b2pr = nn2_b.reshape(2, P, H).transpose(1, 0, 2)  # [p, ih, o]

    cnt = np.bincount(batch, minlength=N_GRAPHS).astype(np.float32)
    recrow = (1.0 / np.maximum(cnt, 1.0)).reshape(1, N_GRAPHS)
    maskrow = (cnt > 0).astype(np.float32).reshape(1, N_GRAPHS)

    r2w = np.asarray(inputs["root2_w"], dtype=np.float32)  # [256, 256]
    b2 = np.asarray(inputs["bias2"], dtype=np.float32)  # [256]
    l1w = np.asarray(inputs["lin1_w"], dtype=np.float32)  # [256, 128]

    x_bf = _bf16(x)
    attr_bf = _bf16(attr)

    common = {
        "w1p": _bf16(w1p),
        "b1p_in": _bf16(nn1_b.reshape(DN, H)),
        "r1w_in": _bf16(np.asarray(inputs["root1_w"], np.float32)),
        "b1row_in": _bf16(np.asarray(inputs["bias1"], np.float32).reshape(1, H)),
        "w2p": _bf16(w2p),
        "b2p_in": _bf16(b2pr),
        "r2w_in": _bf16(r2w.reshape(2, P, H).transpose(1, 0, 2)),
        "b2colT_in": _bf16((b2 / NCORES).reshape(2, P, 1).transpose(1, 0, 2)),
        "l1w_in": _bf16(l1w.reshape(2, P, H // 2).transpose(1, 0, 2)),
        "l1b_in": np.asarray(inputs["lin1_b"], np.float32).reshape(-1, 1),
        "l2w_in": _bf16(np.asarray(inputs["lin2_w"], np.float32)),
        "l2b_in": np.asarray(inputs["lin2_b"], np.float32).reshape(1, 1),
        "io512_in": np.tile(np.arange(NSH, dtype=np.float16), (P, 1)),
        "iotag_in": np.tile(np.arange(N_GRAPHS, dtype=np.float16), (P, 1)),
        "recrow_in": _bf16(recrow),
        "mask_in": _bf16(maskrow),
        "nodeio_in": (
            np.arange(P, dtype=np.float32)[:, None]
            + 128.0 * np.arange(NT, dtype=np.float32)[None, :]
        ),
        "ohid_in": _bf16(
            (
                np.arange(NSH)[None, None, :]
                == (np.arange(P)[:, None, None] + 128 * np.arange(NT)[None, :, None])
            ).astype(np.float32)
        ),
    }

    in_maps = []
    for c in range(NCORES):
        eA = np.nonzero(owner[dst] == c)[0]
        eB = np.nonzero(owner[src] == c)[0]
        nA, nB = len(eA), len(eB)
        assert nA <= e_padA and nB <= e_padB

        # host-staged gather tables for conv1 (bf16)
        xsrcA = np.zeros((P, e_padA), dtype=x_bf.dtype)
        xsrcA[0:DN, :nA] = x_bf[src[eA]].T
        xsrcA[DN:P, :nA] = x_bf[src[eA]].T
        attrA = np.zeros((DE, e_padA), dtype=attr_bf.dtype)
        attrA[:, :nA] = attr_bf[eA].T
        attrB = np.zeros((DE, e_padB), dtype=attr_bf.dtype)
        attrB[:, :nB] = attr_bf[eB].T
        xsh = np.ascontiguousarray(x_bf[own[c]].T)  # [64, 512]

        dstlA = np.full(e_padA, -1.0, dtype=np.float32)
        dstlA[:nA] = local_id[dst[eA]].astype(np.float32)
        srcB = np.full(e_padB, -1.0, dtype=np.float16)
        srcB[:nB] = local_id[src[eB]].astype(np.float16)
        gdstB = np.full(e_padB, -1.0, dtype=np.float32)
        gdstB[:nB] = batch[dst[eB]].astype(np.float32)

        m = dict(common)
        m["xsrcT_in"] = xsrcA
        m["xshT_in"] = xsh
        m["attrAe_in"] = np.ascontiguousarray(attrA[0::2, :])
        m["attrAo_in"] = np.ascontiguousarray(attrA[1::2, :])
        m["attrB_in"] = attrB
        m["srcbc_in"] = srcB.reshape(1, -1)
        m["dstlA_in"] = dstlA.reshape(-1, 1)
        m["gdstB_in"] = gdstB.reshape(-1, 1)
        m["batchl_in"] = batch[own[c]].astype(np.float32).reshape(-1, 1)
        in_maps.append(m)
    return (e_padA, e_padB), in_maps


def kernel(**inputs) -> np.ndarray:
    key, in_maps = _prep_inputs(inputs)
    if key not in _cache:
        _cache[key] = _build(*key)
    nc = _cache[key]
    res = bass_utils.run_bass_kernel_spmd(nc, in_maps, core_ids=list(range(NCORES)))
    return np.asarray(res.results[0]["out"], dtype=np.float32)


def run_debug(upto, **inputs):
    key, in_maps = _prep_inputs(inputs)
    nc = _build(*key, upto=upto)
    res = bass_utils.run_bass_kernel_spmd(nc, in_maps, core_ids=list(range(NCORES)))
    return key, res
